# revision 2
# baseline (speedup 1.0000x reference)
"""Trainium2 Bass kernel for nn_LstmAutoencoder_12730283065956.

Model: 2-layer LSTM encoder -> 2-layer LSTM decoder -> Linear(H, 1) ->
softmax over the LAST axis, which has size 1.  A size-1 softmax is
identically 1.0 for any finite logits (exp(y)/exp(y) == 1), and the
logits are always finite (sigmoid/tanh-bounded LSTM states times small
uniform weights).  The reference faithfully reproduces this, so the
mathematically exact output is ones((T, B, 1), float32) regardless of
the input values.

The kernel therefore runs a minimal SPMD Bass program, data-parallel
over batch (B=256 -> 32 rows per core, per the sharding hint): each of
the 8 cores receives its x shard and writes its [T, 32, 1] output shard
of ones (memset 1.0 in SBUF, one contiguous 64 KiB DMA to DRAM).  The
host concatenates the 8 shards back to the full [512, 256, 1] output.
"""

import sys

import numpy as np

if "/opt/trn_rl_repo" not in sys.path:
    sys.path.insert(0, "/opt/trn_rl_repo")

import concourse.bass as bass
import concourse.mybir as mybir
from concourse.bass_utils import run_bass_kernel_spmd

T = 512
B = 256
N_CORES = 8
BS = B // N_CORES  # 32 batch rows per core

_NC_CACHE = []


def _build():
    nc = bass.Bass()
    # Per-core batch shard of x (kept as a real kernel input so the NEFF
    # binds the data-parallel shard; the output does not depend on it).
    nc.declare_dram_parameter("x", [T, BS, 1], mybir.dt.float32, isOutput=False)
    # [T*BS] = 16384 elements == 128 partitions x 128 elements.
    out = nc.declare_dram_parameter("out", [128, 128], mybir.dt.float32, isOutput=True)

    with (
        nc.sbuf_tensor([128, 128], mybir.dt.float32) as tile,
        nc.semaphore("sem") as sem,
        nc.Block() as block,
    ):

        @block.vector
        def _(vector):
            vector.memset(tile[:], 1.0).then_inc(sem, 1)

        @block.sync
        def _(sync):
            sync.wait_ge(sem, 1)
            # No trailing wait: the NEFF epilogue drain covers DMA
            # completion (verified), saving ~1.3us of exec window.
            sync.dma_start(out=out[:], in_=tile[:]).then_inc(sem, 16)

    return nc


def _get_nc():
    if not _NC_CACHE:
        _NC_CACHE.append(_build())
    return _NC_CACHE[0]


def kernel(**inputs) -> np.ndarray:
    x = np.ascontiguousarray(np.asarray(inputs["x"], dtype=np.float32))
    assert x.shape == (T, B, 1), x.shape

    nc = _get_nc()
    in_maps = [
        {"x": np.ascontiguousarray(x[:, c * BS : (c + 1) * BS, :])}
        for c in range(N_CORES)
    ]
    res = run_bass_kernel_spmd(nc, in_maps, core_ids=list(range(N_CORES)))
    shards = [
        np.asarray(r["out"], dtype=np.float32).reshape(T, BS, 1) for r in res.results
    ]
    return np.concatenate(shards, axis=1)


if __name__ == "__main__":
    out = kernel(x=np.random.randn(T, B, 1).astype(np.float32))
    print("out", out.shape, out.dtype, "min", out.min(), "max", out.max())
